# revision 2
# baseline (speedup 1.0000x reference)
"""GCNII conv kernel v2 for 8 Trainium2 NeuronCores.

Improvements over baseline:
  - dma_gather calls round-robin over 4 SWDGE queues (queue_num=0..3): the
    descriptor generation runs on a different Q7 core pair per queue, giving
    ~3.6x gather throughput (the baseline's single-queue gather was 95% of
    its runtime).
  - Selection matrices S (scatter + norm) are precomputed on host and DMA'd
    in, replacing the per-chunk DVE iota/is_equal build (was 487us busy).
  - S blocks are narrow: each chunk's matmul only covers the dest window its
    edges touch (union across cores to keep one shared program); chunk 0 is
    full-width with start=True so PSUM needs no separate zeroing.
  - Padding indices are -1 and sorted to the tail of each gather call, so the
    Q7 descriptor loop trims them for free (only real edges cost time).
  - bf16 finalize path (x0 add + W_eff matmul).

Structure (as baseline): edges sorted by dest; 128-dest tiles snake-dealt to
cores by edge count; per tile, edges split by source half (int16 indices);
output produced transposed and flipped on host.
"""

import os
import sys

sys.path.insert(0, "/opt/trn_rl_repo")

import numpy as np
import ml_dtypes

bf = ml_dtypes.bfloat16

N = 50000
D = 128
NCORES = 8
NPC = N // NCORES
TPC = (NPC + 127) // 128       # 49 slots per core
NPAD = TPC * 128
NT = (N + 127) // 128          # 391 global dest tiles
HALF = N // 2
ALPHA = 0.1
THETA = 0.5
LAYER = 1
NQ = 4                         # SWDGE queues

_prog_cache = {}
LAST = None


def _wrap16(idx_list):
    w = idx_list.reshape(-1, 16).T.astype(np.int16)
    return np.tile(w, (8, 1))


def _build_program(schedule):
    """schedule: per slot dict(Mlo, Mhi, chunks=[(w0,width),...]) shared by
    all cores; chunk list is lo chunks then hi chunks, chunk 0 full width."""
    import concourse.bacc as bacc
    import concourse.mybir as mybir
    import concourse.tile as tile
    from concourse import library_config

    f32 = mybir.dt.float32
    bf16 = mybir.dt.bfloat16
    i16 = mybir.dt.int16

    IDXC = sum((s["Mlo"] + s["Mhi"]) * 8 for s in schedule)
    STOT = sum(w for s in schedule for (_, w) in s["chunks"])
    NCALLS = sum((1 if s["Mlo"] else 0) + (1 if s["Mhi"] else 0) for s in schedule)

    # greedy queue assignment by earliest-free (schedule sizes as proxy)
    qfree = [0.0] * NQ
    qassign = []
    for s in schedule:
        for M in (s["Mlo"], s["Mhi"]):
            if not M:
                continue
            q = min(range(NQ), key=lambda i: qfree[i])
            qfree[q] += M
            qassign.append(q)

    nc = bacc.Bacc(
        "TRN2", target_bir_lowering=False, debug=False, num_devices=NCORES,
        num_swdge_queues=NQ,
    )
    xlo = nc.dram_tensor("xlo", [HALF, D], bf16, kind="ExternalInput").ap()
    xhi = nc.dram_tensor("xhi", [N - HALF, D], bf16, kind="ExternalInput").ap()
    idx = nc.dram_tensor("idx", [128, IDXC], i16, kind="ExternalInput").ap()
    sm = nc.dram_tensor("sm", [128, STOT], bf16, kind="ExternalInput").ap()
    x0t = nc.dram_tensor("x0t", [D, NPAD], f32, kind="ExternalInput").ap()
    wl = nc.dram_tensor("wl", [D, D], bf16, kind="ExternalInput").ap()
    yt = nc.dram_tensor("yt", [D, NPAD], f32, kind="ExternalOutput").ap()

    with tile.TileContext(nc) as tc:
        with (
            tc.tile_pool(name="persist", bufs=1) as pp,
            tc.tile_pool(name="msgs", bufs=4) as mp,
            tc.tile_pool(name="sm", bufs=2) as sp,
            tc.tile_pool(name="fin", bufs=2) as fp,
            tc.tile_pool(name="ps", bufs=2, space="PSUM") as psp,
            tc.tile_pool(name="py", bufs=2, space="PSUM") as pyp,
        ):
            nc.gpsimd.load_library(library_config.mlp)

            idx_sb = pp.tile([128, IDXC], i16)
            wl_sb = pp.tile([128, 128], bf16)
            # split the idx load so the first tiles' gathers start early
            c1 = min(
                sum((s["Mlo"] + s["Mhi"]) * 8 for s in schedule[:4]), IDXC
            )
            nc.sync.dma_start(idx_sb[:, 0:c1], idx[:, 0:c1])
            if c1 < IDXC:
                nc.sync.dma_start(idx_sb[:, c1:IDXC], idx[:, c1:IDXC])
            nc.sync.dma_start(wl_sb[:], wl[:, :])

            qn = 0
            ioff = 0
            soff = 0
            for t, s in enumerate(schedule):
                Mlo, Mhi = s["Mlo"], s["Mhi"]
                M = Mlo + Mhi
                msgs = mp.tile([128, M, 128], bf16, tag="msgs")
                if t < 4:
                    # first pool rotation: clear stale SBUF so trimmed rows
                    # (S row = 0) never hit NaN * 0 in the PE
                    nc.vector.memset(msgs[:], 0)
                if Mlo:
                    nc.gpsimd.dma_gather(
                        msgs[:, 0:Mlo, :],
                        xlo[:, :],
                        idx_sb[:, ioff : ioff + Mlo * 8],
                        Mlo * 128,
                        Mlo * 128,
                        D,
                        single_packet=False,
                        queue_num=qassign[qn],
                    )
                    qn += 1
                if Mhi:
                    nc.gpsimd.dma_gather(
                        msgs[:, Mlo:M, :],
                        xhi[:, :],
                        idx_sb[:, ioff + Mlo * 8 : ioff + M * 8],
                        Mhi * 128,
                        Mhi * 128,
                        D,
                        single_packet=False,
                        queue_num=qassign[qn],
                    )
                    qn += 1

                swidth = sum(w for (_, w) in s["chunks"])
                st = sp.tile([128, swidth], bf16, tag="sm")
                nc.sync.dma_start(st[:], sm[:, soff : soff + swidth])

                ps = psp.tile([128, 128], f32, space="PSUM", tag="ps")
                cw = 0
                for j, (w0, width) in enumerate(s["chunks"]):
                    nc.tensor.matmul(
                        ps[:, w0 : w0 + width],
                        lhsT=msgs[:, j, :],
                        rhs=st[:, cw : cw + width],
                        start=(j == 0),
                        stop=(j == len(s["chunks"]) - 1),
                        skip_group_check=True,
                    )
                    cw += width

                x0tile = fp.tile([128, 128], f32, tag="x0")
                nc.sync.dma_start(x0tile[:], x0t[:, t * 128 : (t + 1) * 128])
                hT = fp.tile([128, 128], bf16, tag="h")
                nc.vector.tensor_tensor(
                    out=hT[:], in0=ps[:], in1=x0tile[:], op=mybir.AluOpType.add
                )
                yp = pyp.tile([128, 128], f32, space="PSUM", tag="py")
                nc.tensor.matmul(
                    yp[:], lhsT=wl_sb[:], rhs=hT[:], start=True, stop=True
                )
                yo = fp.tile([128, 128], f32, tag="yo")
                nc.vector.tensor_copy(yo[:], yp[:])
                nc.sync.dma_start(yt[:, t * 128 : (t + 1) * 128], yo[:])

                ioff += M * 8
                soff += swidth

    nc.compile()
    return nc


def _preprocess(x, x0, edge_index, norm, W):
    row = np.ascontiguousarray(edge_index[0]).astype(np.int64)
    col = np.ascontiguousarray(edge_index[1]).astype(np.int64)
    norm = np.ascontiguousarray(norm).astype(np.float32)
    x = np.ascontiguousarray(x).astype(np.float32)
    x0 = np.ascontiguousarray(x0).astype(np.float32)
    W = np.ascontiguousarray(W).astype(np.float32)

    order = np.argsort(col, kind="stable")
    rs = row[order]
    cs = col[order]
    ns = (1.0 - ALPHA) * norm[order]

    tstart = np.arange(NT) * 128
    tend = np.minimum(tstart + 128, N)
    e_lo = np.searchsorted(cs, tstart, side="left")
    e_hi = np.searchsorted(cs, tend, side="left")
    cnt = e_hi - e_lo

    # group tiles with similar per-half chunk counts into the same slot to
    # minimize the shared-schedule max padding
    nlo_t = np.array(
        [(rs[e_lo[g] : e_hi[g]] < HALF).sum() for g in range(NT)],
        dtype=np.int64,
    )
    nhi_t = cnt - nlo_t
    order_t = np.lexsort((-nhi_t, -(nlo_t // 128)))
    assign = -np.ones((NCORES, TPC), dtype=np.int64)
    k = 0
    for r in range(TPC):
        picks = order_t[k : k + NCORES]
        k += len(picks)
        cores = range(NCORES) if r % 2 == 0 else range(NCORES - 1, -1, -1)
        for i, c in enumerate(cores):
            if i < len(picks):
                assign[c, r] = picks[i]

    # per (core, slot): lo/hi edge lists (dest-sorted within each half)
    per_ct = {}
    Mlo_ct = np.zeros((NCORES, TPC), dtype=np.int64)
    Mhi_ct = np.zeros((NCORES, TPC), dtype=np.int64)
    for c in range(NCORES):
        for t in range(TPC):
            g = assign[c, t]
            if g < 0:
                per_ct[(c, t)] = None
                continue
            e0, e1 = e_lo[g], e_hi[g]
            r_ = rs[e0:e1]
            dl = (cs[e0:e1] - tstart[g]).astype(np.int64)
            w = ns[e0:e1]
            m = r_ < HALF
            per_ct[(c, t)] = (
                r_[m], dl[m], w[m],
                r_[~m] - HALF, dl[~m], w[~m],
            )
            Mlo_ct[c, t] = -(-int(m.sum()) // 128)
            Mhi_ct[c, t] = -(-int((~m).sum()) // 128)

    Mlo_t = Mlo_ct.max(axis=0)
    Mhi_t = Mhi_ct.max(axis=0)
    empty = (Mlo_t + Mhi_t) == 0
    Mlo_t[empty] = 1

    # shared chunk windows: union over cores of each chunk's dest range
    def chunk_ranges(dl, Mreal):
        outs = []
        for j in range(Mreal):
            seg = dl[j * 128 : (j + 1) * 128]
            if len(seg):
                outs.append((int(seg.min()), int(seg.max()) + 1))
            else:
                outs.append((0, 1))
        return outs

    schedule = []
    for t in range(TPC):
        Mlo, Mhi = int(Mlo_t[t]), int(Mhi_t[t])
        lo_r = [[128, 0] for _ in range(Mlo)]
        hi_r = [[128, 0] for _ in range(Mhi)]
        for c in range(NCORES):
            data = per_ct[(c, t)]
            if data is None:
                continue
            _, dlo, _, _, dhi, _ = data
            for j, (a, b) in enumerate(chunk_ranges(dlo, int(Mlo_ct[c, t]))):
                lo_r[j][0] = min(lo_r[j][0], a)
                lo_r[j][1] = max(lo_r[j][1], b)
            for j, (a, b) in enumerate(chunk_ranges(dhi, int(Mhi_ct[c, t]))):
                hi_r[j][0] = min(hi_r[j][0], a)
                hi_r[j][1] = max(hi_r[j][1], b)
        chunks = []
        for j, (a, b) in enumerate(lo_r + hi_r):
            if j == 0:
                chunks.append((0, 128))
            else:
                if b <= a:
                    a, b = 0, 1
                a = min(a, 127)
                b = min(max(b, a + 1), 128)
                chunks.append((int(a), int(b - a)))
        schedule.append({"Mlo": Mlo, "Mhi": Mhi, "chunks": chunks})

    IDXC = sum((s["Mlo"] + s["Mhi"]) * 8 for s in schedule)
    STOT = sum(w for s in schedule for (_, w) in s["chunks"])

    beta = np.float32(np.log(THETA / LAYER + 1.0))
    W_eff = (1.0 - beta) * np.eye(D, dtype=np.float32) + beta * W
    wl = np.ascontiguousarray(W_eff.T).astype(bf)
    xlo_a = np.ascontiguousarray(x[:HALF]).astype(bf)
    xhi_a = np.ascontiguousarray(x[HALF:]).astype(bf)

    in_maps = []
    for c in range(NCORES):
        idx_a = np.zeros((128, IDXC), dtype=np.int16)
        sm_a = np.zeros((128, STOT), dtype=np.float32)
        x0t = np.zeros((D, NPAD), dtype=np.float32)
        ioff = 0
        soff = 0
        for t in range(TPC):
            s = schedule[t]
            Mlo, Mhi = s["Mlo"], s["Mhi"]
            data = per_ct[(c, t)]
            if data is not None:
                g = assign[c, t]
                sz = int(tend[g] - tstart[g])
                x0t[:, t * 128 : t * 128 + sz] = (
                    ALPHA * x0[tstart[g] : tend[g]]
                ).T
                plo, dlo, wlo, phi, dhi, whi = data
            else:
                plo = dlo = wlo = np.zeros(0)
                phi = dhi = whi = np.zeros(0)
            cw = 0
            for half, (p, dl, wv, M) in enumerate(
                ((plo, dlo, wlo, Mlo), (phi, dhi, whi, Mhi))
            ):
                if M == 0:
                    continue
                n_e = len(p)
                pi = np.zeros(M * 128, dtype=np.int64)
                pi[:n_e] = p
                idx_a[:, ioff : ioff + M * 8] = _wrap16(pi)
                ioff += M * 8
                for j in range(M):
                    cj = j if half == 0 else Mlo + j
                    w0, width = s["chunks"][cj]
                    e0, e1 = j * 128, min((j + 1) * 128, n_e)
                    if e1 > e0:
                        er = np.arange(e0, e1) - e0
                        dcol = dl[e0:e1] - w0
                        sm_a[er, soff + cw + dcol] = wv[e0:e1]
                    cw += width
            soff += cw
        assert ioff == IDXC and soff == STOT, (ioff, IDXC, soff, STOT)
        in_maps.append(
            {
                "xlo": xlo_a, "xhi": xhi_a,
                "idx": idx_a, "sm": sm_a.astype(bf),
                "x0t": x0t, "wl": wl,
            }
        )
    return schedule, in_maps, (assign, tstart, tend)


def kernel(x, x0, edge_index, norm, W):
    global LAST
    from concourse.bass_utils import run_bass_kernel_spmd

    schedule, in_maps, (assign, tstart, tend) = _preprocess(
        x, x0, edge_index, norm, W
    )
    key = tuple((s["Mlo"], s["Mhi"], tuple(s["chunks"])) for s in schedule)
    if key not in _prog_cache:
        _prog_cache[key] = _build_program(schedule)
    nc = _prog_cache[key]

    trace = os.environ.get("KERNEL_TRACE", "0") == "1"
    res = run_bass_kernel_spmd(
        nc,
        in_maps,
        core_ids=list(range(NCORES)),
        trace=trace,
    )
    LAST = res

    y = np.empty((N, D), dtype=np.float32)
    for c in range(NCORES):
        yt = res.results[c]["yt"]
        for t in range(TPC):
            g = assign[c, t]
            if g < 0:
                continue
            sz = int(tend[g] - tstart[g])
            y[tstart[g] : tend[g]] = yt[:, t * 128 : t * 128 + sz].T
    return y


# revision 3
# speedup vs baseline: 1.5413x; 1.5413x over previous
"""GCNII conv kernel v2 for 8 Trainium2 NeuronCores.

Improvements over baseline:
  - dma_gather calls round-robin over 4 SWDGE queues (queue_num=0..3): the
    descriptor generation runs on a different Q7 core pair per queue, giving
    ~3.6x gather throughput (the baseline's single-queue gather was 95% of
    its runtime).
  - Selection matrices S (scatter + norm) are precomputed on host and DMA'd
    in, replacing the per-chunk DVE iota/is_equal build (was 487us busy).
  - S blocks are narrow: each chunk's matmul only covers the dest window its
    edges touch (union across cores to keep one shared program); chunk 0 is
    full-width with start=True so PSUM needs no separate zeroing.
  - Padding slots gather row 0 (idx 0) with zero S entries; runtime count
    trimming (-1 idxs or num_idxs_reg) crashes this stack, so the full padded
    count is gathered.
  - bf16 finalize path (x0 add + W_eff matmul).

Structure (as baseline): edges sorted by dest; 128-dest tiles snake-dealt to
cores by edge count; per tile, edges split by source half (int16 indices);
output produced transposed and flipped on host.
"""

import os
import sys

sys.path.insert(0, "/opt/trn_rl_repo")

import numpy as np
import ml_dtypes

bf = ml_dtypes.bfloat16

N = 50000
D = 128
NCORES = 8
NPC = N // NCORES
TPC = (NPC + 127) // 128       # 49 slots per core
NPAD = TPC * 128
NT = (N + 127) // 128          # 391 global dest tiles
HALF = N // 2
ALPHA = 0.1
THETA = 0.5
LAYER = 1
NQ = 4                         # SWDGE queues

_prog_cache = {}
LAST = None


def _wrap16(idx_list):
    w = idx_list.reshape(-1, 16).T.astype(np.int16)
    return np.tile(w, (8, 1))


def _build_program(schedule):
    """schedule: per slot dict(Mlo, Mhi, chunks=[(w0,width),...]) shared by
    all cores; chunk list is lo chunks then hi chunks, chunk 0 full width."""
    import concourse.bacc as bacc
    import concourse.mybir as mybir
    import concourse.tile as tile
    from concourse import library_config

    f32 = mybir.dt.float32
    bf16 = mybir.dt.bfloat16
    i16 = mybir.dt.int16

    IDXC = sum((s["Mlo"] + s["Mhi"]) * 8 for s in schedule)
    STOT = sum(w for s in schedule for (_, w) in s["chunks"])
    NCALLS = sum((1 if s["Mlo"] else 0) + (1 if s["Mhi"] else 0) for s in schedule)

    # greedy queue assignment by earliest-free (schedule sizes as proxy)
    qfree = [0.0] * NQ
    qassign = []
    for s in schedule:
        for M in (s["Mlo"], s["Mhi"]):
            if not M:
                continue
            q = min(range(NQ), key=lambda i: qfree[i])
            qfree[q] += M
            qassign.append(q)

    nc = bacc.Bacc(
        "TRN2", target_bir_lowering=False, debug=False, num_devices=NCORES,
        num_swdge_queues=NQ,
    )
    xlo = nc.dram_tensor("xlo", [HALF, D], bf16, kind="ExternalInput").ap()
    xhi = nc.dram_tensor("xhi", [N - HALF, D], bf16, kind="ExternalInput").ap()
    idx = nc.dram_tensor("idx", [128, IDXC], i16, kind="ExternalInput").ap()
    sm = nc.dram_tensor("sm", [128, STOT], bf16, kind="ExternalInput").ap()
    x0t = nc.dram_tensor("x0t", [D, NPAD], f32, kind="ExternalInput").ap()
    wl = nc.dram_tensor("wl", [D, D], bf16, kind="ExternalInput").ap()
    yt = nc.dram_tensor("yt", [D, NPAD], f32, kind="ExternalOutput").ap()

    with tile.TileContext(nc) as tc:
        with (
            tc.tile_pool(name="persist", bufs=1) as pp,
            tc.tile_pool(name="msgs", bufs=4) as mp,
            tc.tile_pool(name="sm", bufs=2) as sp,
            tc.tile_pool(name="fin", bufs=2) as fp,
            tc.tile_pool(name="ps", bufs=2, space="PSUM") as psp,
            tc.tile_pool(name="py", bufs=2, space="PSUM") as pyp,
        ):
            nc.gpsimd.load_library(library_config.mlp)

            idx_sb = pp.tile([128, IDXC], i16)
            wl_sb = pp.tile([128, 128], bf16)
            # split the idx load so the first tiles' gathers start early
            c1 = min(
                sum((s["Mlo"] + s["Mhi"]) * 8 for s in schedule[:4]), IDXC
            )
            nc.sync.dma_start(idx_sb[:, 0:c1], idx[:, 0:c1])
            if c1 < IDXC:
                nc.sync.dma_start(idx_sb[:, c1:IDXC], idx[:, c1:IDXC])
            nc.sync.dma_start(wl_sb[:], wl[:, :])

            qn = 0
            ioff = 0
            soff = 0
            for t, s in enumerate(schedule):
                Mlo, Mhi = s["Mlo"], s["Mhi"]
                M = Mlo + Mhi
                msgs = mp.tile([128, M, 128], bf16, tag="msgs")
                if t < 4:
                    # first pool rotation: clear stale SBUF so trimmed rows
                    # (S row = 0) never hit NaN * 0 in the PE
                    nc.vector.memset(msgs[:], 0)
                if Mlo:
                    nc.gpsimd.dma_gather(
                        msgs[:, 0:Mlo, :],
                        xlo[:, :],
                        idx_sb[:, ioff : ioff + Mlo * 8],
                        Mlo * 128,
                        Mlo * 128,
                        D,
                        single_packet=False,
                        queue_num=qassign[qn],
                    )
                    qn += 1
                if Mhi:
                    nc.gpsimd.dma_gather(
                        msgs[:, Mlo:M, :],
                        xhi[:, :],
                        idx_sb[:, ioff + Mlo * 8 : ioff + M * 8],
                        Mhi * 128,
                        Mhi * 128,
                        D,
                        single_packet=False,
                        queue_num=qassign[qn],
                    )
                    qn += 1

                swidth = sum(w for (_, w) in s["chunks"])
                st = sp.tile([128, swidth], bf16, tag="sm")
                nc.sync.dma_start(st[:], sm[:, soff : soff + swidth])

                ps = psp.tile([128, 128], f32, space="PSUM", tag="ps")
                cw = 0
                for j, (w0, width) in enumerate(s["chunks"]):
                    nc.tensor.matmul(
                        ps[:, w0 : w0 + width],
                        lhsT=msgs[:, j, :],
                        rhs=st[:, cw : cw + width],
                        start=(j == 0),
                        stop=(j == len(s["chunks"]) - 1),
                        skip_group_check=True,
                    )
                    cw += width

                x0tile = fp.tile([128, 128], f32, tag="x0")
                nc.sync.dma_start(x0tile[:], x0t[:, t * 128 : (t + 1) * 128])
                hT = fp.tile([128, 128], bf16, tag="h")
                nc.vector.tensor_tensor(
                    out=hT[:], in0=ps[:], in1=x0tile[:], op=mybir.AluOpType.add
                )
                yp = pyp.tile([128, 128], f32, space="PSUM", tag="py")
                nc.tensor.matmul(
                    yp[:], lhsT=wl_sb[:], rhs=hT[:], start=True, stop=True
                )
                yo = fp.tile([128, 128], f32, tag="yo")
                nc.vector.tensor_copy(yo[:], yp[:])
                nc.sync.dma_start(yt[:, t * 128 : (t + 1) * 128], yo[:])

                ioff += M * 8
                soff += swidth

    nc.compile()
    return nc


def _preprocess(x, x0, edge_index, norm, W):
    row = np.ascontiguousarray(edge_index[0]).astype(np.int64)
    col = np.ascontiguousarray(edge_index[1]).astype(np.int64)
    norm = np.ascontiguousarray(norm).astype(np.float32)
    x = np.ascontiguousarray(x).astype(np.float32)
    x0 = np.ascontiguousarray(x0).astype(np.float32)
    W = np.ascontiguousarray(W).astype(np.float32)

    order = np.argsort(col, kind="stable")
    rs = row[order]
    cs = col[order]
    ns = (1.0 - ALPHA) * norm[order]

    tstart = np.arange(NT) * 128
    tend = np.minimum(tstart + 128, N)
    e_lo = np.searchsorted(cs, tstart, side="left")
    e_hi = np.searchsorted(cs, tend, side="left")
    cnt = e_hi - e_lo

    # group tiles with similar per-half chunk counts into the same slot to
    # minimize the shared-schedule max padding
    nlo_t = np.array(
        [(rs[e_lo[g] : e_hi[g]] < HALF).sum() for g in range(NT)],
        dtype=np.int64,
    )
    nhi_t = cnt - nlo_t
    order_t = np.lexsort((-nhi_t, -(nlo_t // 128)))
    assign = -np.ones((NCORES, TPC), dtype=np.int64)
    k = 0
    for r in range(TPC):
        picks = order_t[k : k + NCORES]
        k += len(picks)
        cores = range(NCORES) if r % 2 == 0 else range(NCORES - 1, -1, -1)
        for i, c in enumerate(cores):
            if i < len(picks):
                assign[c, r] = picks[i]

    # per (core, slot): lo/hi edge lists (dest-sorted within each half)
    per_ct = {}
    Mlo_ct = np.zeros((NCORES, TPC), dtype=np.int64)
    Mhi_ct = np.zeros((NCORES, TPC), dtype=np.int64)
    for c in range(NCORES):
        for t in range(TPC):
            g = assign[c, t]
            if g < 0:
                per_ct[(c, t)] = None
                continue
            e0, e1 = e_lo[g], e_hi[g]
            r_ = rs[e0:e1]
            dl = (cs[e0:e1] - tstart[g]).astype(np.int64)
            w = ns[e0:e1]
            m = r_ < HALF
            per_ct[(c, t)] = (
                r_[m], dl[m], w[m],
                r_[~m] - HALF, dl[~m], w[~m],
            )
            Mlo_ct[c, t] = -(-int(m.sum()) // 128)
            Mhi_ct[c, t] = -(-int((~m).sum()) // 128)

    Mlo_t = Mlo_ct.max(axis=0)
    Mhi_t = Mhi_ct.max(axis=0)
    empty = (Mlo_t + Mhi_t) == 0
    Mlo_t[empty] = 1

    # shared chunk windows: union over cores of each chunk's dest range
    def chunk_ranges(dl, Mreal):
        outs = []
        for j in range(Mreal):
            seg = dl[j * 128 : (j + 1) * 128]
            if len(seg):
                outs.append((int(seg.min()), int(seg.max()) + 1))
            else:
                outs.append((0, 1))
        return outs

    schedule = []
    for t in range(TPC):
        Mlo, Mhi = int(Mlo_t[t]), int(Mhi_t[t])
        lo_r = [[128, 0] for _ in range(Mlo)]
        hi_r = [[128, 0] for _ in range(Mhi)]
        for c in range(NCORES):
            data = per_ct[(c, t)]
            if data is None:
                continue
            _, dlo, _, _, dhi, _ = data
            for j, (a, b) in enumerate(chunk_ranges(dlo, int(Mlo_ct[c, t]))):
                lo_r[j][0] = min(lo_r[j][0], a)
                lo_r[j][1] = max(lo_r[j][1], b)
            for j, (a, b) in enumerate(chunk_ranges(dhi, int(Mhi_ct[c, t]))):
                hi_r[j][0] = min(hi_r[j][0], a)
                hi_r[j][1] = max(hi_r[j][1], b)
        chunks = []
        for j, (a, b) in enumerate(lo_r + hi_r):
            if j == 0:
                chunks.append((0, 128))
            else:
                if b <= a:
                    a, b = 0, 1
                a = min(a, 127)
                b = min(max(b, a + 1), 128)
                chunks.append((int(a), int(b - a)))
        schedule.append({"Mlo": Mlo, "Mhi": Mhi, "chunks": chunks})

    IDXC = sum((s["Mlo"] + s["Mhi"]) * 8 for s in schedule)
    STOT = sum(w for s in schedule for (_, w) in s["chunks"])

    beta = np.float32(np.log(THETA / LAYER + 1.0))
    W_eff = (1.0 - beta) * np.eye(D, dtype=np.float32) + beta * W
    wl = np.ascontiguousarray(W_eff.T).astype(bf)
    xlo_a = np.ascontiguousarray(x[:HALF]).astype(bf)
    xhi_a = np.ascontiguousarray(x[HALF:]).astype(bf)

    in_maps = []
    for c in range(NCORES):
        idx_a = np.zeros((128, IDXC), dtype=np.int16)
        sm_a = np.zeros((128, STOT), dtype=np.float32)
        x0t = np.zeros((D, NPAD), dtype=np.float32)
        ioff = 0
        soff = 0
        for t in range(TPC):
            s = schedule[t]
            Mlo, Mhi = s["Mlo"], s["Mhi"]
            data = per_ct[(c, t)]
            if data is not None:
                g = assign[c, t]
                sz = int(tend[g] - tstart[g])
                x0t[:, t * 128 : t * 128 + sz] = (
                    ALPHA * x0[tstart[g] : tend[g]]
                ).T
                plo, dlo, wlo, phi, dhi, whi = data
            else:
                plo = dlo = wlo = np.zeros(0)
                phi = dhi = whi = np.zeros(0)
            cw = 0
            for half, (p, dl, wv, M) in enumerate(
                ((plo, dlo, wlo, Mlo), (phi, dhi, whi, Mhi))
            ):
                if M == 0:
                    continue
                n_e = len(p)
                pi = np.zeros(M * 128, dtype=np.int64)
                pi[:n_e] = p
                idx_a[:, ioff : ioff + M * 8] = _wrap16(pi)
                ioff += M * 8
                for j in range(M):
                    cj = j if half == 0 else Mlo + j
                    w0, width = s["chunks"][cj]
                    e0, e1 = j * 128, min((j + 1) * 128, n_e)
                    if e1 > e0:
                        er = np.arange(e0, e1) - e0
                        dcol = dl[e0:e1] - w0
                        sm_a[er, soff + cw + dcol] = wv[e0:e1]
                    cw += width
            soff += cw
        assert ioff == IDXC and soff == STOT, (ioff, IDXC, soff, STOT)
        in_maps.append(
            {
                "xlo": xlo_a, "xhi": xhi_a,
                "idx": idx_a, "sm": sm_a.astype(bf),
                "x0t": x0t, "wl": wl,
            }
        )
    return schedule, in_maps, (assign, tstart, tend)


def kernel(x, x0, edge_index, norm, W):
    global LAST
    from concourse.bass_utils import run_bass_kernel_spmd

    schedule, in_maps, (assign, tstart, tend) = _preprocess(
        x, x0, edge_index, norm, W
    )
    key = tuple((s["Mlo"], s["Mhi"], tuple(s["chunks"])) for s in schedule)
    if key not in _prog_cache:
        _prog_cache[key] = _build_program(schedule)
    nc = _prog_cache[key]

    trace = os.environ.get("KERNEL_TRACE", "0") == "1"
    res = run_bass_kernel_spmd(
        nc,
        in_maps,
        core_ids=list(range(NCORES)),
        trace=trace,
    )
    LAST = res

    y = np.empty((N, D), dtype=np.float32)
    for c in range(NCORES):
        yt = res.results[c]["yt"]
        for t in range(TPC):
            g = assign[c, t]
            if g < 0:
                continue
            sz = int(tend[g] - tstart[g])
            y[tstart[g] : tend[g]] = yt[:, t * 128 : t * 128 + sz].T
    return y
